# revision 15
# baseline (speedup 1.0000x reference)
"""Trainium2 Bass kernel for NaiveAttentionModule.

Reference computation (B=2, S=2048, E=1024, H=16, D=64):
    qkv = x @ w_qkv.T            -> q, k, v per head
    attn = softmax(causal(q k^T / sqrt(D)))
    out  = (attn @ v) merged across heads
    y    = out @ w_out.T

Sharding (8 cores, one NEFF, SPMD):
    Head-parallel: core c owns heads {2c, 2c+1}. Every core receives the full
    activation x and the weight slices for its heads, computes q/k/v for its
    two heads, runs causal attention for its 4 (batch, head) instances, and
    applies the slice of the output projection that contracts over its heads'
    feature columns. Per-core results are partial sums of the final output;
    the host sums the 8 partials (the cross-head reduction of out @ w_out.T).

    All per-core differences enter through input data only (weight slices),
    so one instruction stream serves all cores, and attention causality is
    exploited identically on every core (each (b, h) instance has the same
    triangle structure).

Device pipeline per core:
    1. x [4096, 1024] f32 --gpsimd cast DMA--> bf16 --HWDGE xbar transpose-->
       XT (embed on partitions).  DMA-only transpose; no compute engines.
    2. QKV projection (bf16 matmuls, N=512): Q^T, K^T (d on partitions) and
       V^T; V^T is then PE-transposed to natural V [s, d] and augmented with a
       ones column so the attention denominator falls out of the attn @ V'
       matmul (row 64), avoiding any cross-partition reduction.
    3. Causal attention per (b, h): scores^T tiles [128k, 512q] = K^T-block^T
       @ Q^T-group (fp32r), exp via ScalarE (scale=1/sqrt(D) folded in, no max
       subtraction -- scores ~ N(0,1), exp is safe in f32), causal mask via
       gpsimd affine_select on diagonal tiles only, attn @ V' accumulated in
       PSUM over k-blocks.
    4. Output projection partial: y_c = outT_c^T @ w_out^T[e-slice] (fp32r).
"""

import functools

import numpy as np
import ml_dtypes

import concourse.bass as bass
import concourse.tile as tile
from concourse import bacc, mybir
from concourse.bass_utils import run_bass_kernel_spmd
from concourse.masks import make_identity

N_CORES = 8
B, S, E = 2, 2048, 1024
H, D = 16, 64
BS = B * S            # 4096 flattened (b, s) rows
HPC = H // N_CORES    # 2 heads per core
SCALE = 1.0 / D ** 0.5

F32 = mybir.dt.float32
F32R = mybir.dt.float32r
BF16 = mybir.dt.bfloat16

KB = S // 128         # 16 k-blocks of 128 per instance
QG = S // 512         # 4 q-groups of 512 per instance


def _r(ap):
    return ap.bitcast(F32R)


def _build():
    nc = bacc.Bacc("TRN2", target_bir_lowering=False, debug=False,
                   enable_asserts=False, num_devices=N_CORES)

    x = nc.dram_tensor("xt", [128, 8, BS], BF16, kind="ExternalInput").ap()
    wqk = nc.dram_tensor("wqk", [E, 256], BF16, kind="ExternalInput").ap()
    wv = nc.dram_tensor("wv", [E, 128], BF16, kind="ExternalInput").ap()
    wout = nc.dram_tensor("wout", [128, E], BF16, kind="ExternalInput").ap()
    ones = nc.dram_tensor("ones", [128, KB], BF16, kind="ExternalInput").ap()
    y = nc.dram_tensor("y", [BS, E], F32, kind="ExternalOutput").ap()

    with tile.TileContext(nc) as tc:
        _body(tc, y, x, wqk, wv, wout, ones)
    nc.compile()
    return nc


def _body(tc, y, x, wqk, wv, wout, ones, debug=False):
    nc = tc.nc
    with (
        tc.tile_pool(name="xt", bufs=1) as xt_pool,
        tc.tile_pool(name="w", bufs=1) as w_pool,
        tc.tile_pool(name="qk", bufs=1) as qk_pool,
        tc.tile_pool(name="vsb", bufs=1) as v_pool,
        tc.tile_pool(name="at", bufs=4) as at_pool,
        tc.tile_pool(name="small", bufs=1) as small_pool,
        tc.tile_pool(name="rc", bufs=2) as rc_pool,
        tc.tile_pool(name="yo", bufs=4) as y_pool,
        tc.tile_pool(name="ps2", bufs=2, space="PSUM") as ps2,
        tc.tile_pool(name="pssc", bufs=2, space="PSUM") as pssc,
        tc.tile_pool(name="psacc", bufs=4, space="PSUM") as psacc,
    ):
        # ---- constants -------------------------------------------------
        ident = small_pool.tile([128, 128], F32, tag="ident")
        make_identity(nc, ident[:])

        # ---- weights ---------------------------------------------------
        wqk_sb = w_pool.tile([128, 8, 256], BF16, tag="wqk")
        nc.sync.dma_start(wqk_sb[:], wqk.rearrange("(a p) f -> p a f", p=128))
        wv_sb = w_pool.tile([128, 8, 128], BF16, tag="wv")
        nc.scalar.dma_start(wv_sb[:], wv.rearrange("(a p) f -> p a f", p=128))
        wout_sb = w_pool.tile([128, E], BF16, tag="wout")
        nc.scalar.dma_start(wout_sb[:], wout[:])

        # ---- stage 1: load host-pretransposed X^T (bf16) ---------------
        # xt_chunks[ci]: [128 e-part, 8 e-block, 1024 bs-cols] bf16, covering
        # bs columns [ci*1024, (ci+1)*1024).  xt[p, eb, c] = x[c, eb*128 + p].
        xt_chunks = []
        for ci in range(4):
            xt_chunks.append(xt_pool.tile([128, 8, 1024], BF16, tag=f"xt{ci}", name=f"xt{ci}"))
            nc.sync.dma_start(xt_chunks[ci][:],
                              x[:, :, ci * 1024:(ci + 1) * 1024])

        def xt_slice(g):  # bs-column group g (512 wide), e-block et
            ci, off = g // 2, (g % 2) * 512

            def f(et):
                return xt_chunks[ci][:, et, off:off + 512]
            return f

        # ---- stage 2: QKV projection ----------------------------------
        # q_t[b] / k_t[b]: [128 (h0 rows 0-63 | h1 rows 64-127), 2048 s] f32
        q_t = [qk_pool.tile([128, S], BF16, tag=f"q{b}", name=f"q{b}") for b in range(B)]
        k_t = [qk_pool.tile([128, S], BF16, tag=f"k{b}", name=f"k{b}") for b in range(B)]
        vt_t = [qk_pool.tile([128, S], F32, tag=f"vt{b}", name=f"vt{b}") for b in range(B)]

        for fb, dest in ((0, q_t), (1, k_t)):
            for g in range(BS // 512):
                ps = ps2.tile([128, 512], F32, tag="mm2")
                sl = xt_slice(g)
                for et in range(8):
                    nc.tensor.matmul(ps[:], wqk_sb[:, et, fb * 128:fb * 128 + 128],
                                     sl(et), start=(et == 0), stop=(et == 7))
                b, scol = g // 4, (g % 4) * 512
                nc.vector.tensor_copy(dest[b][:, scol:scol + 512], ps[:])
        for g in range(BS // 512):
            ps = ps2.tile([128, 512], F32, tag="mm2")
            sl = xt_slice(g)
            for et in range(8):
                nc.tensor.matmul(ps[:], wv_sb[:, et, :], sl(et),
                                 start=(et == 0), stop=(et == 7))
            b, scol = g // 4, (g % 4) * 512
            nc.vector.tensor_copy(vt_t[b][:, scol:scol + 512], ps[:])

        # V natural layout with ones column: v_sb[b][h]: [128 s, 16 kb, 65]
        v_sb = [[v_pool.tile([128, KB, 65], BF16, tag=f"v{b}{h}", name=f"v{b}{h}")
                 for h in range(HPC)] for b in range(B)]
        for b in range(B):
            for h in range(HPC):
                nc.gpsimd.dma_start(v_sb[b][h][:, :, 64], ones[:])
            for kb in range(KB):
                ps = ps2.tile([128, 128], F32, tag="mm2")
                nc.tensor.transpose(ps[:], vt_t[b][:, kb * 128:(kb + 1) * 128],
                                    ident[:])
                for h in range(HPC):
                    nc.vector.tensor_copy(v_sb[b][h][:, kb, 0:64],
                                          ps[:, h * 64:h * 64 + 64])

        # ---- stage 3: causal attention --------------------------------
        # qg-outer, kb-inner; h0/h1 interleaved so their 64-row score
        # matmuls occupy disjoint PE row-groups and execute concurrently.
        outT = qk_pool.tile([128, BS], BF16, tag="outT")
        dbg = None
        if debug:
            dbg = {"xt0": xt_chunks[0], "q0": q_t[0], "v00": v_sb[0][0],
                   "outT": outT, "at00": rc_pool.tile([128, 512], BF16, tag="atd", name="atd")}
        for b in range(B):
            q_ap = [q_t[b][h * 64:h * 64 + 64, :] for h in range(HPC)]
            k_ap = [k_t[b][h * 64:h * 64 + 64, :] for h in range(HPC)]
            for qg in range(QG):
                acc = [psacc.tile([65, 512], F32, tag="acc", name="acc")
                       for _ in range(HPC)]
                nkb = 4 * qg + 4
                for kb in range(nkb):
                    ats = []
                    for h in range(HPC):
                        sc = pssc.tile([128, 512], F32, tag="sc")
                        nc.tensor.matmul(
                            sc[:],
                            k_ap[h][:, kb * 128:(kb + 1) * 128],
                            q_ap[h][:, qg * 512:(qg + 1) * 512],
                            start=True, stop=True,
                            tile_position=(h * 64, 0))
                        at = at_pool.tile([128, 512], BF16, tag="at")
                        nc.scalar.activation(at[:], sc[:],
                                             mybir.ActivationFunctionType.Exp,
                                             scale=SCALE)
                        if qg == kb // 4:
                            nc.gpsimd.affine_select(
                                out=at[:], in_=at[:], pattern=[[1, 512]],
                                compare_op=mybir.AluOpType.is_ge, fill=0.0,
                                base=qg * 512 - kb * 128,
                                channel_multiplier=-1)
                        if debug and b == 0 and h == 0 and kb == 0 and qg == 0:
                            nc.vector.tensor_copy(dbg["at00"][:], at[:])
                        ats.append(at)
                    for h in range(HPC):
                        nc.tensor.matmul(acc[h][:], v_sb[b][h][:, kb, :],
                                         ats[h][:], start=(kb == 0),
                                         stop=(kb == nkb - 1))
                for h in range(HPC):
                    den = rc_pool.tile([1, 512], F32, tag="den")
                    nc.vector.tensor_copy(den[:], acc[h][64:65, :])
                    db = rc_pool.tile([64, 512], F32, tag="db")
                    nc.gpsimd.partition_broadcast(db[:], den[:])
                    rb = rc_pool.tile([64, 512], F32, tag="rb")
                    nc.vector.reciprocal(rb[:], db[:])
                    dst = outT[h * 64:h * 64 + 64,
                               b * S + qg * 512:b * S + (qg + 1) * 512]
                    nc.vector.tensor_mul(dst, acc[h][0:64, :], rb[:])

        # ---- stage 4: output projection partial -----------------------
        for bst in range(BS // 128):
            for ft in range(2):
                ps = ps2.tile([128, 512], F32, tag="mm2")
                nc.tensor.matmul(ps[:], outT[:, bst * 128:(bst + 1) * 128],
                                 wout_sb[:, ft * 512:(ft + 1) * 512],
                                 start=True, stop=True)
                ysb = y_pool.tile([128, 512], F32, tag="ysb")
                nc.vector.tensor_copy(ysb[:], ps[:])
                nc.sync.dma_start(
                    y[bst * 128:(bst + 1) * 128, ft * 512:(ft + 1) * 512],
                    ysb[:])
        return dbg


@functools.lru_cache(maxsize=1)
def _get_nc():
    return _build()


def _shard_inputs(x, w_qkv, w_out):
    xb = np.asarray(x, dtype=np.float32).reshape(BS, E).astype(ml_dtypes.bfloat16)
    xt = np.ascontiguousarray(xb.reshape(BS, 8, 128).transpose(2, 1, 0))
    w_qkv = np.asarray(w_qkv, dtype=np.float32)
    w_out = np.asarray(w_out, dtype=np.float32)
    in_maps = []
    for c in range(N_CORES):
        h0 = HPC * c
        rows_q = np.arange(h0 * D, (h0 + HPC) * D)
        wq = w_qkv[rows_q, :]                    # [128, 1024]
        wk = w_qkv[E + rows_q, :]                # [128, 1024]
        wv_ = w_qkv[2 * E + rows_q, :]           # [128, 1024]
        wqk_c = np.ascontiguousarray(
            np.concatenate([wq, wk], axis=0).T).astype(ml_dtypes.bfloat16)
        wv_c = np.ascontiguousarray(wv_.T).astype(ml_dtypes.bfloat16)
        wout_c = np.ascontiguousarray(w_out[:, rows_q].T).astype(ml_dtypes.bfloat16)
        in_maps.append({"xt": xt, "wqk": wqk_c, "wv": wv_c, "wout": wout_c,
                        "ones": np.ones((128, KB), ml_dtypes.bfloat16)})
    return in_maps


def kernel(x, w_qkv, w_out):
    nc = _get_nc()
    in_maps = _shard_inputs(x, w_qkv, w_out)
    res = run_bass_kernel_spmd(nc, in_maps, core_ids=list(range(N_CORES)))
    y = res.results[0]["y"].astype(np.float64)
    for c in range(1, N_CORES):
        y = y + res.results[c]["y"]
    return y.astype(np.float32).reshape(B, S, E)


# revision 16
# speedup vs baseline: 1.0119x; 1.0119x over previous
"""Trainium2 Bass kernel for NaiveAttentionModule.

Reference computation (B=2, S=2048, E=1024, H=16, D=64):
    qkv = x @ w_qkv.T            -> q, k, v per head
    attn = softmax(causal(q k^T / sqrt(D)))
    out  = (attn @ v) merged across heads
    y    = out @ w_out.T

Sharding (8 cores, one NEFF, SPMD):
    Head-parallel: core c owns heads {2c, 2c+1}. Every core receives the full
    activation x and the weight slices for its heads, computes q/k/v for its
    two heads, runs causal attention for its 4 (batch, head) instances, and
    applies the slice of the output projection that contracts over its heads'
    feature columns. Per-core results are partial sums of the final output;
    the host sums the 8 partials (the cross-head reduction of out @ w_out.T).

    All per-core differences enter through input data only (weight slices),
    so one instruction stream serves all cores, and attention causality is
    exploited identically on every core (each (b, h) instance has the same
    triangle structure).

Device pipeline per core:
    1. x [4096, 1024] f32 --gpsimd cast DMA--> bf16 --HWDGE xbar transpose-->
       XT (embed on partitions).  DMA-only transpose; no compute engines.
    2. QKV projection (bf16 matmuls, N=512): Q^T, K^T (d on partitions) and
       V^T; V^T is then PE-transposed to natural V [s, d] and augmented with a
       ones column so the attention denominator falls out of the attn @ V'
       matmul (row 64), avoiding any cross-partition reduction.
    3. Causal attention per (b, h): scores^T tiles [128k, 512q] = K^T-block^T
       @ Q^T-group (fp32r), exp via ScalarE (scale=1/sqrt(D) folded in, no max
       subtraction -- scores ~ N(0,1), exp is safe in f32), causal mask via
       gpsimd affine_select on diagonal tiles only, attn @ V' accumulated in
       PSUM over k-blocks.
    4. Output projection partial: y_c = outT_c^T @ w_out^T[e-slice] (fp32r).
"""

import functools

import numpy as np
import ml_dtypes

import concourse.bass as bass
import concourse.tile as tile
from concourse import bacc, mybir
from concourse.bass_utils import run_bass_kernel_spmd
from concourse.masks import make_identity

N_CORES = 8
B, S, E = 2, 2048, 1024
H, D = 16, 64
BS = B * S            # 4096 flattened (b, s) rows
HPC = H // N_CORES    # 2 heads per core
SCALE = 1.0 / D ** 0.5

F32 = mybir.dt.float32
F32R = mybir.dt.float32r
BF16 = mybir.dt.bfloat16

KB = S // 128         # 16 k-blocks of 128 per instance
QG = S // 512         # 4 q-groups of 512 per instance


def _r(ap):
    return ap.bitcast(F32R)


def _build():
    nc = bacc.Bacc("TRN2", target_bir_lowering=False, debug=False,
                   enable_asserts=False, num_devices=N_CORES)

    x = nc.dram_tensor("xt", [128, 8, BS], BF16, kind="ExternalInput").ap()
    wqk = nc.dram_tensor("wqk", [E, 256], BF16, kind="ExternalInput").ap()
    wv = nc.dram_tensor("wv", [E, 128], BF16, kind="ExternalInput").ap()
    wout = nc.dram_tensor("wout", [128, E], BF16, kind="ExternalInput").ap()
    ones = nc.dram_tensor("ones", [128, KB], BF16, kind="ExternalInput").ap()
    y = nc.dram_tensor("y", [BS, E], F32, kind="ExternalOutput").ap()

    with tile.TileContext(nc) as tc:
        _body(tc, y, x, wqk, wv, wout, ones)
    nc.compile()
    return nc


def _body(tc, y, x, wqk, wv, wout, ones, debug=False):
    nc = tc.nc
    with (
        tc.tile_pool(name="xt", bufs=1) as xt_pool,
        tc.tile_pool(name="w", bufs=1) as w_pool,
        tc.tile_pool(name="qk", bufs=1) as qk_pool,
        tc.tile_pool(name="vsb", bufs=1) as v_pool,
        tc.tile_pool(name="at", bufs=8) as at_pool,
        tc.tile_pool(name="small", bufs=1) as small_pool,
        tc.tile_pool(name="rc", bufs=3) as rc_pool,
        tc.tile_pool(name="yo", bufs=4) as y_pool,
        tc.tile_pool(name="ps2", bufs=2, space="PSUM") as ps2,
        tc.tile_pool(name="pssc", bufs=2, space="PSUM") as pssc,
        tc.tile_pool(name="psacc", bufs=4, space="PSUM") as psacc,
    ):
        # ---- constants -------------------------------------------------
        ident = small_pool.tile([128, 128], F32, tag="ident")
        make_identity(nc, ident[:])

        # ---- weights ---------------------------------------------------
        wqk_sb = w_pool.tile([128, 8, 256], BF16, tag="wqk")
        nc.sync.dma_start(wqk_sb[:], wqk.rearrange("(a p) f -> p a f", p=128))
        wv_sb = w_pool.tile([128, 8, 128], BF16, tag="wv")
        nc.scalar.dma_start(wv_sb[:], wv.rearrange("(a p) f -> p a f", p=128))
        wout_sb = w_pool.tile([128, E], BF16, tag="wout")
        nc.scalar.dma_start(wout_sb[:], wout[:])

        # ---- stage 1: load host-pretransposed X^T (bf16) ---------------
        # xt_chunks[ci]: [128 e-part, 8 e-block, 1024 bs-cols] bf16, covering
        # bs columns [ci*1024, (ci+1)*1024).  xt[p, eb, c] = x[c, eb*128 + p].
        xt_chunks = []
        for ci in range(4):
            xt_chunks.append(xt_pool.tile([128, 8, 1024], BF16, tag=f"xt{ci}", name=f"xt{ci}"))
            nc.sync.dma_start(xt_chunks[ci][:],
                              x[:, :, ci * 1024:(ci + 1) * 1024])

        def xt_slice(g):  # bs-column group g (512 wide), e-block et
            ci, off = g // 2, (g % 2) * 512

            def f(et):
                return xt_chunks[ci][:, et, off:off + 512]
            return f

        # ---- stage 2: QKV projection ----------------------------------
        # q_t[b] / k_t[b]: [128 (h0 rows 0-63 | h1 rows 64-127), 2048 s] f32
        q_t = [qk_pool.tile([128, S], BF16, tag=f"q{b}", name=f"q{b}") for b in range(B)]
        k_t = [qk_pool.tile([128, S], BF16, tag=f"k{b}", name=f"k{b}") for b in range(B)]
        vt_t = [qk_pool.tile([128, S], F32, tag=f"vt{b}", name=f"vt{b}") for b in range(B)]

        for fb, dest in ((0, q_t), (1, k_t)):
            for g in range(BS // 512):
                ps = ps2.tile([128, 512], F32, tag="mm2")
                sl = xt_slice(g)
                for et in range(8):
                    nc.tensor.matmul(ps[:], wqk_sb[:, et, fb * 128:fb * 128 + 128],
                                     sl(et), start=(et == 0), stop=(et == 7))
                b, scol = g // 4, (g % 4) * 512
                nc.vector.tensor_copy(dest[b][:, scol:scol + 512], ps[:])
        for g in range(BS // 512):
            ps = ps2.tile([128, 512], F32, tag="mm2")
            sl = xt_slice(g)
            for et in range(8):
                nc.tensor.matmul(ps[:], wv_sb[:, et, :], sl(et),
                                 start=(et == 0), stop=(et == 7))
            b, scol = g // 4, (g % 4) * 512
            nc.vector.tensor_copy(vt_t[b][:, scol:scol + 512], ps[:])

        # V natural layout with ones column: v_sb[b][h]: [128 s, 16 kb, 65]
        v_sb = [[v_pool.tile([128, KB, 65], BF16, tag=f"v{b}{h}", name=f"v{b}{h}")
                 for h in range(HPC)] for b in range(B)]
        for b in range(B):
            for h in range(HPC):
                nc.gpsimd.dma_start(v_sb[b][h][:, :, 64], ones[:])
            for kb in range(KB):
                ps = ps2.tile([128, 128], F32, tag="mm2")
                nc.tensor.transpose(ps[:], vt_t[b][:, kb * 128:(kb + 1) * 128],
                                    ident[:])
                for h in range(HPC):
                    nc.vector.tensor_copy(v_sb[b][h][:, kb, 0:64],
                                          ps[:, h * 64:h * 64 + 64])

        # ---- stage 3: causal attention --------------------------------
        # qg-outer, kb-inner; h0/h1 interleaved so their 64-row score
        # matmuls occupy disjoint PE row-groups and execute concurrently.
        outT = qk_pool.tile([128, BS], BF16, tag="outT")
        dbg = None
        if debug:
            dbg = {"xt0": xt_chunks[0], "q0": q_t[0], "v00": v_sb[0][0],
                   "outT": outT, "at00": rc_pool.tile([128, 512], BF16, tag="atd", name="atd")}
        for b in range(B):
            q_ap = [q_t[b][h * 64:h * 64 + 64, :] for h in range(HPC)]
            k_ap = [k_t[b][h * 64:h * 64 + 64, :] for h in range(HPC)]
            for qg in range(QG):
                acc = [psacc.tile([65, 512], F32, tag="acc", name="acc")
                       for _ in range(HPC)]
                nkb = 4 * qg + 4
                for kb in range(nkb):
                    ats = []
                    for h in range(HPC):
                        sc = pssc.tile([128, 512], F32, tag="sc")
                        nc.tensor.matmul(
                            sc[:],
                            k_ap[h][:, kb * 128:(kb + 1) * 128],
                            q_ap[h][:, qg * 512:(qg + 1) * 512],
                            start=True, stop=True,
                            tile_position=(h * 64, 0))
                        at = at_pool.tile([128, 512], BF16, tag="at")
                        nc.scalar.activation(at[:], sc[:],
                                             mybir.ActivationFunctionType.Exp,
                                             scale=SCALE)
                        if qg == kb // 4:
                            nc.gpsimd.affine_select(
                                out=at[:], in_=at[:], pattern=[[1, 512]],
                                compare_op=mybir.AluOpType.is_ge, fill=0.0,
                                base=qg * 512 - kb * 128,
                                channel_multiplier=-1)
                        if debug and b == 0 and h == 0 and kb == 0 and qg == 0:
                            nc.vector.tensor_copy(dbg["at00"][:], at[:])
                        ats.append(at)
                    for h in range(HPC):
                        nc.tensor.matmul(acc[h][:], v_sb[b][h][:, kb, :],
                                         ats[h][:], start=(kb == 0),
                                         stop=(kb == nkb - 1))
                for h in range(HPC):
                    den = rc_pool.tile([1, 512], F32, tag="den")
                    nc.vector.tensor_copy(den[:], acc[h][64:65, :])
                    db = rc_pool.tile([64, 512], F32, tag="db")
                    nc.gpsimd.partition_broadcast(db[:], den[:])
                    rb = rc_pool.tile([64, 512], F32, tag="rb")
                    nc.vector.reciprocal(rb[:], db[:])
                    dst = outT[h * 64:h * 64 + 64,
                               b * S + qg * 512:b * S + (qg + 1) * 512]
                    nc.vector.tensor_mul(dst, acc[h][0:64, :], rb[:])

        # ---- stage 4: output projection partial -----------------------
        for bst in range(BS // 128):
            for ft in range(2):
                ps = ps2.tile([128, 512], F32, tag="mm2")
                nc.tensor.matmul(ps[:], outT[:, bst * 128:(bst + 1) * 128],
                                 wout_sb[:, ft * 512:(ft + 1) * 512],
                                 start=True, stop=True)
                ysb = y_pool.tile([128, 512], F32, tag="ysb")
                nc.vector.tensor_copy(ysb[:], ps[:])
                nc.sync.dma_start(
                    y[bst * 128:(bst + 1) * 128, ft * 512:(ft + 1) * 512],
                    ysb[:])
        return dbg


@functools.lru_cache(maxsize=1)
def _get_nc():
    return _build()


def _shard_inputs(x, w_qkv, w_out):
    xb = np.asarray(x, dtype=np.float32).reshape(BS, E).astype(ml_dtypes.bfloat16)
    xt = np.ascontiguousarray(xb.reshape(BS, 8, 128).transpose(2, 1, 0))
    w_qkv = np.asarray(w_qkv, dtype=np.float32)
    w_out = np.asarray(w_out, dtype=np.float32)
    in_maps = []
    for c in range(N_CORES):
        h0 = HPC * c
        rows_q = np.arange(h0 * D, (h0 + HPC) * D)
        wq = w_qkv[rows_q, :]                    # [128, 1024]
        wk = w_qkv[E + rows_q, :]                # [128, 1024]
        wv_ = w_qkv[2 * E + rows_q, :]           # [128, 1024]
        wqk_c = np.ascontiguousarray(
            np.concatenate([wq, wk], axis=0).T).astype(ml_dtypes.bfloat16)
        wv_c = np.ascontiguousarray(wv_.T).astype(ml_dtypes.bfloat16)
        wout_c = np.ascontiguousarray(w_out[:, rows_q].T).astype(ml_dtypes.bfloat16)
        in_maps.append({"xt": xt, "wqk": wqk_c, "wv": wv_c, "wout": wout_c,
                        "ones": np.ones((128, KB), ml_dtypes.bfloat16)})
    return in_maps


def kernel(x, w_qkv, w_out):
    nc = _get_nc()
    in_maps = _shard_inputs(x, w_qkv, w_out)
    res = run_bass_kernel_spmd(nc, in_maps, core_ids=list(range(N_CORES)))
    y = res.results[0]["y"].astype(np.float64)
    for c in range(1, N_CORES):
        y = y + res.results[c]["y"]
    return y.astype(np.float32).reshape(B, S, E)


# revision 17
# speedup vs baseline: 1.1956x; 1.1815x over previous
"""Trainium2 Bass kernel for NaiveAttentionModule.

Reference computation (B=2, S=2048, E=1024, H=16, D=64):
    qkv = x @ w_qkv.T            -> q, k, v per head
    attn = softmax(causal(q k^T / sqrt(D)))
    out  = (attn @ v) merged across heads
    y    = out @ w_out.T

Sharding (8 cores, one NEFF, SPMD):
    Head-parallel: core c owns heads {2c, 2c+1}. Every core receives the full
    activation x and the weight slices for its heads, computes q/k/v for its
    two heads, runs causal attention for its 4 (batch, head) instances, and
    applies the slice of the output projection that contracts over its heads'
    feature columns. Per-core results are partial sums of the final output;
    the host sums the 8 partials (the cross-head reduction of out @ w_out.T).

    All per-core differences enter through input data only (weight slices),
    so one instruction stream serves all cores, and attention causality is
    exploited identically on every core (each (b, h) instance has the same
    triangle structure).

Device pipeline per core:
    1. x [4096, 1024] f32 --gpsimd cast DMA--> bf16 --HWDGE xbar transpose-->
       XT (embed on partitions).  DMA-only transpose; no compute engines.
    2. QKV projection (bf16 matmuls, N=512): Q^T, K^T (d on partitions) and
       V^T; V^T is then PE-transposed to natural V [s, d] and augmented with a
       ones column so the attention denominator falls out of the attn @ V'
       matmul (row 64), avoiding any cross-partition reduction.
    3. Causal attention per (b, h): scores^T tiles [128k, 512q] = K^T-block^T
       @ Q^T-group (fp32r), exp via ScalarE (scale=1/sqrt(D) folded in, no max
       subtraction -- scores ~ N(0,1), exp is safe in f32), causal mask via
       gpsimd affine_select on diagonal tiles only, attn @ V' accumulated in
       PSUM over k-blocks.
    4. Output projection partial: y_c = outT_c^T @ w_out^T[e-slice] (fp32r).
"""

import functools

import numpy as np
import ml_dtypes

import concourse.bass as bass
import concourse.tile as tile
from concourse import bacc, mybir
from concourse.bass_utils import run_bass_kernel_spmd
from concourse.masks import make_identity

N_CORES = 8
B, S, E = 2, 2048, 1024
H, D = 16, 64
BS = B * S            # 4096 flattened (b, s) rows
HPC = H // N_CORES    # 2 heads per core
SCALE = 1.0 / D ** 0.5

F32 = mybir.dt.float32
F32R = mybir.dt.float32r
BF16 = mybir.dt.bfloat16

KB = S // 128         # 16 k-blocks of 128 per instance
QG = S // 512         # 4 q-groups of 512 per instance


def _r(ap):
    return ap.bitcast(F32R)


def _build():
    nc = bacc.Bacc("TRN2", target_bir_lowering=False, debug=False,
                   enable_asserts=False, num_devices=N_CORES)

    x = nc.dram_tensor("xt", [128, 8, BS], BF16, kind="ExternalInput").ap()
    wqk = nc.dram_tensor("wqk", [E, 256], BF16, kind="ExternalInput").ap()
    wv = nc.dram_tensor("wv", [E, 128], BF16, kind="ExternalInput").ap()
    wout = nc.dram_tensor("wout", [128, E], BF16, kind="ExternalInput").ap()
    ones = nc.dram_tensor("ones", [128, KB], BF16, kind="ExternalInput").ap()
    y = nc.dram_tensor("y", [BS, E], F32, kind="ExternalOutput").ap()

    with tile.TileContext(nc) as tc:
        _body(tc, y, x, wqk, wv, wout, ones)
    nc.compile()
    return nc


def _body(tc, y, x, wqk, wv, wout, ones, debug=False):
    nc = tc.nc
    with (
        tc.tile_pool(name="xt", bufs=1) as xt_pool,
        tc.tile_pool(name="w", bufs=1) as w_pool,
        tc.tile_pool(name="qk", bufs=1) as qk_pool,
        tc.tile_pool(name="vsb", bufs=1) as v_pool,
        tc.tile_pool(name="at", bufs=8) as at_pool,
        tc.tile_pool(name="small", bufs=1) as small_pool,
        tc.tile_pool(name="rc", bufs=3) as rc_pool,
        tc.tile_pool(name="yo", bufs=4) as y_pool,
        tc.tile_pool(name="ps2", bufs=2, space="PSUM") as ps2,
        tc.tile_pool(name="pssc", bufs=2, space="PSUM") as pssc,
        tc.tile_pool(name="psacc", bufs=4, space="PSUM") as psacc,
    ):
        # ---- constants -------------------------------------------------
        ident = small_pool.tile([128, 128], F32, tag="ident")
        make_identity(nc, ident[:])

        # ---- weights ---------------------------------------------------
        wqk_sb = w_pool.tile([128, 8, 256], BF16, tag="wqk")
        nc.sync.dma_start(wqk_sb[:], wqk.rearrange("(a p) f -> p a f", p=128))
        wv_sb = w_pool.tile([128, 8, 128], BF16, tag="wv")
        nc.scalar.dma_start(wv_sb[:], wv.rearrange("(a p) f -> p a f", p=128))
        wout_sb = w_pool.tile([128, E], BF16, tag="wout")
        nc.scalar.dma_start(wout_sb[:], wout[:])

        # ---- stage 1: load host-pretransposed X^T (bf16) ---------------
        # xt_chunks[ci]: [128 e-part, 8 e-block, 1024 bs-cols] bf16, covering
        # bs columns [ci*1024, (ci+1)*1024).  xt[p, eb, c] = x[c, eb*128 + p].
        xt_chunks = []
        for ci in range(8):
            xt_chunks.append(xt_pool.tile([128, 8, 512], BF16, tag=f"xt{ci}", name=f"xt{ci}"))
            nc.sync.dma_start(xt_chunks[ci][:],
                              x[:, :, ci * 512:(ci + 1) * 512])

        def xt_slice(g):  # bs-column group g (512 wide), e-block et
            def f(et):
                return xt_chunks[g][:, et, :]
            return f

        # ---- stage 2: QKV projection ----------------------------------
        # q_t[b] / k_t[b]: [128 (h0 rows 0-63 | h1 rows 64-127), 2048 s] f32
        q_t = [[qk_pool.tile([128, S], BF16, tag=f"q{b}{h}", name=f"q{b}{h}")
                for h in range(HPC)] for b in range(B)]
        for b in range(B):
            for h in range(HPC):
                nc.vector.memset(q_t[b][h][:], 0.0)
        k_t = [qk_pool.tile([128, S], BF16, tag=f"k{b}", name=f"k{b}") for b in range(B)]
        vt_t = [qk_pool.tile([128, S], F32, tag=f"vt{b}", name=f"vt{b}") for b in range(B)]

        for fb in (0, 1):
            for g in range(BS // 512):
                ps = ps2.tile([128, 512], F32, tag="mm2")
                sl = xt_slice(g)
                for et in range(8):
                    nc.tensor.matmul(ps[:], wqk_sb[:, et, fb * 128:fb * 128 + 128],
                                     sl(et), start=(et == 0), stop=(et == 7))
                b, scol = g // 4, (g % 4) * 512
                if fb == 0:
                    for h in range(HPC):
                        nc.vector.tensor_copy(
                            q_t[b][h][h * 64:h * 64 + 64, scol:scol + 512],
                            ps[h * 64:h * 64 + 64, :])
                else:
                    nc.vector.tensor_copy(k_t[b][:, scol:scol + 512], ps[:])
        for g in range(BS // 512):
            ps = ps2.tile([128, 512], F32, tag="mm2")
            sl = xt_slice(g)
            for et in range(8):
                nc.tensor.matmul(ps[:], wv_sb[:, et, :], sl(et),
                                 start=(et == 0), stop=(et == 7))
            b, scol = g // 4, (g % 4) * 512
            nc.vector.tensor_copy(vt_t[b][:, scol:scol + 512], ps[:])

        # V natural layout with ones column: v_sb[b][h]: [128 s, 16 kb, 65]
        v_sb = [[v_pool.tile([128, KB, 65], BF16, tag=f"v{b}{h}", name=f"v{b}{h}")
                 for h in range(HPC)] for b in range(B)]
        for b in range(B):
            for h in range(HPC):
                nc.gpsimd.dma_start(v_sb[b][h][:, :, 64], ones[:])
            for kb in range(KB):
                ps = ps2.tile([128, 128], F32, tag="mm2")
                nc.tensor.transpose(ps[:], vt_t[b][:, kb * 128:(kb + 1) * 128],
                                    ident[:])
                for h in range(HPC):
                    nc.vector.tensor_copy(v_sb[b][h][:, kb, 0:64],
                                          ps[:, h * 64:h * 64 + 64])

        # ---- stage 3: causal attention --------------------------------
        # qg-outer, kb-inner; h0/h1 interleaved so their 64-row score
        # matmuls occupy disjoint PE row-groups and execute concurrently.
        outT = qk_pool.tile([128, BS], BF16, tag="outT")
        dbg = None
        if debug:
            dbg = {"xt0": xt_chunks[0], "q0": q_t[0][0], "v00": v_sb[0][0],
                   "outT": outT, "at00": rc_pool.tile([128, 512], BF16, tag="atd", name="atd")}
        for b in range(B):
            q_ap = [q_t[b][h] for h in range(HPC)]
            for qg in range(QG):
                acc = [psacc.tile([65, 512], F32, tag="acc", name="acc")
                       for _ in range(HPC)]
                nkb = 4 * qg + 4
                for kb in range(nkb):
                    ats = []
                    for h in range(HPC):
                        sc = pssc.tile([128, 512], F32, tag="sc")
                        nc.tensor.matmul(
                            sc[:],
                            k_t[b][:, kb * 128:(kb + 1) * 128],
                            q_ap[h][:, qg * 512:(qg + 1) * 512],
                            start=True, stop=True)
                        at = at_pool.tile([128, 512], BF16, tag="at")
                        nc.scalar.activation(at[:], sc[:],
                                             mybir.ActivationFunctionType.Exp,
                                             scale=SCALE)
                        if qg == kb // 4:
                            nc.gpsimd.affine_select(
                                out=at[:], in_=at[:], pattern=[[1, 512]],
                                compare_op=mybir.AluOpType.is_ge, fill=0.0,
                                base=qg * 512 - kb * 128,
                                channel_multiplier=-1)
                        if debug and b == 0 and h == 0 and kb == 0 and qg == 0:
                            nc.vector.tensor_copy(dbg["at00"][:], at[:])
                        ats.append(at)
                    for h in range(HPC):
                        nc.tensor.matmul(acc[h][:], v_sb[b][h][:, kb, :],
                                         ats[h][:], start=(kb == 0),
                                         stop=(kb == nkb - 1))
                for h in range(HPC):
                    den = rc_pool.tile([1, 512], F32, tag="den")
                    nc.vector.tensor_copy(den[:], acc[h][64:65, :])
                    db = rc_pool.tile([64, 512], F32, tag="db")
                    nc.gpsimd.partition_broadcast(db[:], den[:])
                    rb = rc_pool.tile([64, 512], F32, tag="rb")
                    nc.vector.reciprocal(rb[:], db[:])
                    dst = outT[h * 64:h * 64 + 64,
                               b * S + qg * 512:b * S + (qg + 1) * 512]
                    nc.vector.tensor_mul(dst, acc[h][0:64, :], rb[:])

        # ---- stage 4: output projection partial -----------------------
        for bst in range(BS // 128):
            for ft in range(2):
                ps = ps2.tile([128, 512], F32, tag="mm2")
                nc.tensor.matmul(ps[:], outT[:, bst * 128:(bst + 1) * 128],
                                 wout_sb[:, ft * 512:(ft + 1) * 512],
                                 start=True, stop=True)
                ysb = y_pool.tile([128, 512], F32, tag="ysb")
                nc.vector.tensor_copy(ysb[:], ps[:])
                nc.sync.dma_start(
                    y[bst * 128:(bst + 1) * 128, ft * 512:(ft + 1) * 512],
                    ysb[:])
        return dbg


@functools.lru_cache(maxsize=1)
def _get_nc():
    return _build()


def _shard_inputs(x, w_qkv, w_out):
    xb = np.asarray(x, dtype=np.float32).reshape(BS, E).astype(ml_dtypes.bfloat16)
    xt = np.ascontiguousarray(xb.reshape(BS, 8, 128).transpose(2, 1, 0))
    w_qkv = np.asarray(w_qkv, dtype=np.float32)
    w_out = np.asarray(w_out, dtype=np.float32)
    in_maps = []
    for c in range(N_CORES):
        h0 = HPC * c
        rows_q = np.arange(h0 * D, (h0 + HPC) * D)
        wq = w_qkv[rows_q, :]                    # [128, 1024]
        wk = w_qkv[E + rows_q, :]                # [128, 1024]
        wv_ = w_qkv[2 * E + rows_q, :]           # [128, 1024]
        wqk_c = np.ascontiguousarray(
            np.concatenate([wq, wk], axis=0).T).astype(ml_dtypes.bfloat16)
        wv_c = np.ascontiguousarray(wv_.T).astype(ml_dtypes.bfloat16)
        wout_c = np.ascontiguousarray(w_out[:, rows_q].T).astype(ml_dtypes.bfloat16)
        in_maps.append({"xt": xt, "wqk": wqk_c, "wv": wv_c, "wout": wout_c,
                        "ones": np.ones((128, KB), ml_dtypes.bfloat16)})
    return in_maps


def kernel(x, w_qkv, w_out):
    nc = _get_nc()
    in_maps = _shard_inputs(x, w_qkv, w_out)
    res = run_bass_kernel_spmd(nc, in_maps, core_ids=list(range(N_CORES)))
    y = res.results[0]["y"].astype(np.float64)
    for c in range(1, N_CORES):
        y = y + res.results[c]["y"]
    return y.astype(np.float32).reshape(B, S, E)


# revision 18
# speedup vs baseline: 1.4809x; 1.2386x over previous
"""Trainium2 Bass kernel for NaiveAttentionModule.

Reference computation (B=2, S=2048, E=1024, H=16, D=64):
    qkv = x @ w_qkv.T            -> q, k, v per head
    attn = softmax(causal(q k^T / sqrt(D)))
    out  = (attn @ v) merged across heads
    y    = out @ w_out.T

Sharding (8 cores, one NEFF, SPMD):
    Head-parallel: core c owns heads {2c, 2c+1}. Every core receives the full
    activation x and the weight slices for its heads, computes q/k/v for its
    two heads, runs causal attention for its 4 (batch, head) instances, and
    applies the slice of the output projection that contracts over its heads'
    feature columns. Per-core results are partial sums of the final output;
    the host sums the 8 partials (the cross-head reduction of out @ w_out.T).

    All per-core differences enter through input data only (weight slices),
    so one instruction stream serves all cores, and attention causality is
    exploited identically on every core (each (b, h) instance has the same
    triangle structure).

Device pipeline per core:
    1. x [4096, 1024] f32 --gpsimd cast DMA--> bf16 --HWDGE xbar transpose-->
       XT (embed on partitions).  DMA-only transpose; no compute engines.
    2. QKV projection (bf16 matmuls, N=512): Q^T, K^T (d on partitions) and
       V^T; V^T is then PE-transposed to natural V [s, d] and augmented with a
       ones column so the attention denominator falls out of the attn @ V'
       matmul (row 64), avoiding any cross-partition reduction.
    3. Causal attention per (b, h): scores^T tiles [128k, 512q] = K^T-block^T
       @ Q^T-group (fp32r), exp via ScalarE (scale=1/sqrt(D) folded in, no max
       subtraction -- scores ~ N(0,1), exp is safe in f32), causal mask via
       gpsimd affine_select on diagonal tiles only, attn @ V' accumulated in
       PSUM over k-blocks.
    4. Output projection partial: y_c = outT_c^T @ w_out^T[e-slice] (fp32r).
"""

import functools

import numpy as np
import ml_dtypes

import concourse.bass as bass
import concourse.tile as tile
from concourse import bacc, mybir
from concourse.bass_utils import run_bass_kernel_spmd
from concourse.masks import make_identity

N_CORES = 8
B, S, E = 2, 2048, 1024
H, D = 16, 64
BS = B * S            # 4096 flattened (b, s) rows
HPC = H // N_CORES    # 2 heads per core
SCALE = 1.0 / D ** 0.5

F32 = mybir.dt.float32
F32R = mybir.dt.float32r
BF16 = mybir.dt.bfloat16

KB = S // 128         # 16 k-blocks of 128 per instance
QG = S // 512         # 4 q-groups of 512 per instance


def _r(ap):
    return ap.bitcast(F32R)


def _build():
    nc = bacc.Bacc("TRN2", target_bir_lowering=False, debug=False,
                   enable_asserts=False, num_devices=N_CORES)

    x = nc.dram_tensor("xt", [8, 128, 8, 512], BF16, kind="ExternalInput").ap()
    wqk = nc.dram_tensor("wqk", [E, 256], BF16, kind="ExternalInput").ap()
    wv = nc.dram_tensor("wv", [E, 128], BF16, kind="ExternalInput").ap()
    wout = nc.dram_tensor("wout", [128, E], BF16, kind="ExternalInput").ap()
    ones = nc.dram_tensor("ones", [128, KB], BF16, kind="ExternalInput").ap()
    y = nc.dram_tensor("y", [BS, E], F32, kind="ExternalOutput").ap()

    with tile.TileContext(nc) as tc:
        _body(tc, y, x, wqk, wv, wout, ones)
    nc.compile()
    return nc


def _body(tc, y, x, wqk, wv, wout, ones, debug=False):
    nc = tc.nc
    with (
        tc.tile_pool(name="xt", bufs=1) as xt_pool,
        tc.tile_pool(name="w", bufs=1) as w_pool,
        tc.tile_pool(name="qk", bufs=1) as qk_pool,
        tc.tile_pool(name="vsb", bufs=1) as v_pool,
        tc.tile_pool(name="at", bufs=8) as at_pool,
        tc.tile_pool(name="small", bufs=1) as small_pool,
        tc.tile_pool(name="rc", bufs=3) as rc_pool,
        tc.tile_pool(name="yo", bufs=4) as y_pool,
        tc.tile_pool(name="ps2", bufs=2, space="PSUM") as ps2,
        tc.tile_pool(name="pssc", bufs=2, space="PSUM") as pssc,
        tc.tile_pool(name="psacc", bufs=4, space="PSUM") as psacc,
    ):
        # ---- constants -------------------------------------------------
        ident = small_pool.tile([128, 128], F32, tag="ident")
        make_identity(nc, ident[:])

        # ---- weights ---------------------------------------------------
        wqk_sb = w_pool.tile([128, 8, 256], BF16, tag="wqk")
        nc.sync.dma_start(wqk_sb[:], wqk.rearrange("(a p) f -> p a f", p=128))
        wv_sb = w_pool.tile([128, 8, 128], BF16, tag="wv")
        nc.scalar.dma_start(wv_sb[:], wv.rearrange("(a p) f -> p a f", p=128))
        wout_sb = w_pool.tile([128, E], BF16, tag="wout")
        nc.scalar.dma_start(wout_sb[:], wout[:])

        # ---- stage 1: load host-pretransposed X^T (bf16) ---------------
        # xt_chunks[ci]: [128 e-part, 8 e-block, 1024 bs-cols] bf16, covering
        # bs columns [ci*1024, (ci+1)*1024).  xt[p, eb, c] = x[c, eb*128 + p].
        xt_chunks = []
        for ci in range(8):
            xt_chunks.append(xt_pool.tile([128, 8, 512], BF16, tag=f"xt{ci}", name=f"xt{ci}"))
            nc.sync.dma_start(xt_chunks[ci][:], x[ci])

        def xt_slice(g):  # bs-column group g (512 wide), e-block et
            def f(et):
                return xt_chunks[g][:, et, :]
            return f

        # ---- stage 2: QKV projection ----------------------------------
        # q_t[b] / k_t[b]: [128 (h0 rows 0-63 | h1 rows 64-127), 2048 s] f32
        q_t = [[qk_pool.tile([128, S], BF16, tag=f"q{b}{h}", name=f"q{b}{h}")
                for h in range(HPC)] for b in range(B)]
        for b in range(B):
            for h in range(HPC):
                nc.vector.memset(q_t[b][h][:], 0.0)
        k_t = [qk_pool.tile([128, S], BF16, tag=f"k{b}", name=f"k{b}") for b in range(B)]
        vt_t = [qk_pool.tile([128, S], F32, tag=f"vt{b}", name=f"vt{b}") for b in range(B)]

        for fb in (0, 1):
            for g in range(BS // 512):
                ps = ps2.tile([128, 512], F32, tag="mm2")
                sl = xt_slice(g)
                for et in range(8):
                    nc.tensor.matmul(ps[:], wqk_sb[:, et, fb * 128:fb * 128 + 128],
                                     sl(et), start=(et == 0), stop=(et == 7))
                b, scol = g // 4, (g % 4) * 512
                if fb == 0:
                    for h in range(HPC):
                        nc.vector.tensor_copy(
                            q_t[b][h][h * 64:h * 64 + 64, scol:scol + 512],
                            ps[h * 64:h * 64 + 64, :])
                else:
                    nc.vector.tensor_copy(k_t[b][:, scol:scol + 512], ps[:])
        for g in range(BS // 512):
            ps = ps2.tile([128, 512], F32, tag="mm2")
            sl = xt_slice(g)
            for et in range(8):
                nc.tensor.matmul(ps[:], wv_sb[:, et, :], sl(et),
                                 start=(et == 0), stop=(et == 7))
            b, scol = g // 4, (g % 4) * 512
            nc.vector.tensor_copy(vt_t[b][:, scol:scol + 512], ps[:])

        # V natural layout with ones column: v_sb[b][h]: [128 s, 16 kb, 65]
        v_sb = [[v_pool.tile([128, KB, 65], BF16, tag=f"v{b}{h}", name=f"v{b}{h}")
                 for h in range(HPC)] for b in range(B)]
        for b in range(B):
            for h in range(HPC):
                nc.gpsimd.dma_start(v_sb[b][h][:, :, 64], ones[:])
            for kb in range(KB):
                ps = ps2.tile([128, 128], F32, tag="mm2")
                nc.tensor.transpose(ps[:], vt_t[b][:, kb * 128:(kb + 1) * 128],
                                    ident[:])
                for h in range(HPC):
                    nc.vector.tensor_copy(v_sb[b][h][:, kb, 0:64],
                                          ps[:, h * 64:h * 64 + 64])

        # ---- stage 3: causal attention --------------------------------
        # qg-outer, kb-inner; h0/h1 interleaved so their 64-row score
        # matmuls occupy disjoint PE row-groups and execute concurrently.
        outT = qk_pool.tile([128, BS], BF16, tag="outT")
        dbg = None
        if debug:
            dbg = {"xt0": xt_chunks[0], "q0": q_t[0][0], "v00": v_sb[0][0],
                   "outT": outT, "at00": rc_pool.tile([128, 512], BF16, tag="atd", name="atd")}
        for b in range(B):
            q_ap = [q_t[b][h] for h in range(HPC)]
            for qg in range(QG):
                acc = [psacc.tile([65, 512], F32, tag="acc", name="acc")
                       for _ in range(HPC)]
                nkb = 4 * qg + 4
                for kb in range(nkb):
                    ats = []
                    for h in range(HPC):
                        sc = pssc.tile([128, 512], F32, tag="sc")
                        nc.tensor.matmul(
                            sc[:],
                            k_t[b][:, kb * 128:(kb + 1) * 128],
                            q_ap[h][:, qg * 512:(qg + 1) * 512],
                            start=True, stop=True)
                        at = at_pool.tile([128, 512], BF16, tag="at")
                        nc.scalar.activation(at[:], sc[:],
                                             mybir.ActivationFunctionType.Exp,
                                             scale=SCALE)
                        if qg == kb // 4:
                            nc.gpsimd.affine_select(
                                out=at[:], in_=at[:], pattern=[[1, 512]],
                                compare_op=mybir.AluOpType.is_ge, fill=0.0,
                                base=qg * 512 - kb * 128,
                                channel_multiplier=-1)
                        if debug and b == 0 and h == 0 and kb == 0 and qg == 0:
                            nc.vector.tensor_copy(dbg["at00"][:], at[:])
                        ats.append(at)
                    for h in range(HPC):
                        nc.tensor.matmul(acc[h][:], v_sb[b][h][:, kb, :],
                                         ats[h][:], start=(kb == 0),
                                         stop=(kb == nkb - 1))
                for h in range(HPC):
                    den = rc_pool.tile([1, 512], F32, tag="den")
                    nc.vector.tensor_copy(den[:], acc[h][64:65, :])
                    db = rc_pool.tile([64, 512], F32, tag="db")
                    nc.gpsimd.partition_broadcast(db[:], den[:])
                    rb = rc_pool.tile([64, 512], F32, tag="rb")
                    nc.vector.reciprocal_approx_fast(rb[:], db[:])
                    dst = outT[h * 64:h * 64 + 64,
                               b * S + qg * 512:b * S + (qg + 1) * 512]
                    nc.vector.tensor_mul(dst, acc[h][0:64, :], rb[:])

        # ---- stage 4: output projection partial -----------------------
        for bst in range(BS // 128):
            for ft in range(2):
                ps = ps2.tile([128, 512], F32, tag="mm2")
                nc.tensor.matmul(ps[:], outT[:, bst * 128:(bst + 1) * 128],
                                 wout_sb[:, ft * 512:(ft + 1) * 512],
                                 start=True, stop=True)
                ysb = y_pool.tile([128, 512], F32, tag="ysb")
                nc.vector.tensor_copy(ysb[:], ps[:])
                nc.sync.dma_start(
                    y[bst * 128:(bst + 1) * 128, ft * 512:(ft + 1) * 512],
                    ysb[:])
        return dbg


@functools.lru_cache(maxsize=1)
def _get_nc():
    return _build()


def _shard_inputs(x, w_qkv, w_out):
    xb = np.asarray(x, dtype=np.float32).reshape(BS, E).astype(ml_dtypes.bfloat16)
    xt = np.ascontiguousarray(
        xb.reshape(8, 512, 8, 128).transpose(0, 3, 2, 1))
    w_qkv = np.asarray(w_qkv, dtype=np.float32)
    w_out = np.asarray(w_out, dtype=np.float32)
    in_maps = []
    for c in range(N_CORES):
        h0 = HPC * c
        rows_q = np.arange(h0 * D, (h0 + HPC) * D)
        wq = w_qkv[rows_q, :]                    # [128, 1024]
        wk = w_qkv[E + rows_q, :]                # [128, 1024]
        wv_ = w_qkv[2 * E + rows_q, :]           # [128, 1024]
        wqk_c = np.ascontiguousarray(
            np.concatenate([wq, wk], axis=0).T).astype(ml_dtypes.bfloat16)
        wv_c = np.ascontiguousarray(wv_.T).astype(ml_dtypes.bfloat16)
        wout_c = np.ascontiguousarray(w_out[:, rows_q].T).astype(ml_dtypes.bfloat16)
        in_maps.append({"xt": xt, "wqk": wqk_c, "wv": wv_c, "wout": wout_c,
                        "ones": np.ones((128, KB), ml_dtypes.bfloat16)})
    return in_maps


def kernel(x, w_qkv, w_out):
    nc = _get_nc()
    in_maps = _shard_inputs(x, w_qkv, w_out)
    res = run_bass_kernel_spmd(nc, in_maps, core_ids=list(range(N_CORES)))
    y = res.results[0]["y"].astype(np.float64)
    for c in range(1, N_CORES):
        y = y + res.results[c]["y"]
    return y.astype(np.float32).reshape(B, S, E)
